# revision 4
# baseline (speedup 1.0000x reference)
"""GNN message passing on 8 Trainium2 NeuronCores — v6.

dst-sharded slot grid (49 windows x 128 dsts per core), one-hot matmul
aggregation with the linearity trick (aggregate raw x rows, apply W once
per window). Engine assignment tuned so the f32 HBM gather (222us/rep,
the hard floor) is the only DMA besides outputs:

  * gathers: f32 rows (512B descriptors; sub-512B ones RMW-penalize),
    4 SWDGE queues, adjacent chunks on disjoint queue pairs,
  * DVE: builds bf16 one-hot sel tiles (iota == dloc) per 128-edge tile,
  * scalar engine: casts gathered f32 messages to bf16, plus PSUM
    evacuation and relu,
  * TensorE: bf16 aggregation matmuls + f32 per-window W matmul.
"""

import numpy as np

P = 128
D = 128
N_NODES = 50000
N_CORES = 8
NW = 50
CG = 5
S_SPLIT = 31000
LO_ROWS = S_SPLIT + 1
HI_ROWS = N_NODES - S_SPLIT + 1

_NC_CACHE = {}


def _build_nc(t_lo, t_hi, nw=NW, cg=CG, lo_rows=LO_ROWS, hi_rows=HI_ROWS,
              bench_reps=1, parts="all", dma_scratch=16384, nq=4,
              single_packet=False, tables_internal=False):
    key = (t_lo, t_hi, nw, cg, bench_reps, parts, nq, single_packet,
           tables_internal)
    if key in _NC_CACHE:
        return _NC_CACHE[key]

    import concourse.bacc as bacc
    import concourse.mybir as mybir
    import concourse.tile as tile
    from concourse import library_config

    assert nw % cg == 0
    nch = nw // cg
    nmm = t_lo + t_hi
    n_lo = nw * t_lo * P
    n_hi = nw * t_hi * P

    nc = bacc.Bacc(
        "TRN2", target_bir_lowering=False, debug=False, num_swdge_queues=nq,
        dynamic_dma_scratch_size=dma_scratch,
    )
    f32 = mybir.dt.float32
    bf16 = mybir.dt.bfloat16
    tkind = "Internal" if tables_internal else "ExternalInput"
    tbl_lo = nc.dram_tensor("tbl_lo", [lo_rows, D], f32, kind=tkind)
    tbl_hi = nc.dram_tensor("tbl_hi", [hi_rows, D], f32, kind=tkind)
    idx_lo = nc.dram_tensor(
        "idx_lo", [P, n_lo // 16], mybir.dt.int16, kind="ExternalInput"
    )
    idx_hi = nc.dram_tensor(
        "idx_hi", [P, n_hi // 16], mybir.dt.int16, kind="ExternalInput"
    )
    dlh = nc.dram_tensor("dlh", [P, nw * nmm], bf16, kind="ExternalInput")
    wmat = nc.dram_tensor("wmat", [D, D], f32, kind="ExternalInput")
    iota = nc.dram_tensor("iota", [P, nmm * P], bf16, kind="ExternalInput")
    out = nc.dram_tensor("out", [nw * P, D], f32, kind="ExternalOutput")

    with tile.TileContext(nc) as tc:
        nc.gpsimd.load_library(library_config.mlp)
        with (
            tc.tile_pool(name="const", bufs=1) as cpool,
            tc.tile_pool(name="stga", bufs=3) as sapool,
            tc.tile_pool(name="stgb", bufs=3) as sbpool,
            tc.tile_pool(name="msga", bufs=2) as apool,
            tc.tile_pool(name="msgb", bufs=2) as bpool,
            tc.tile_pool(name="sel", bufs=8) as spool,
            tc.tile_pool(name="agg", bufs=4) as gpool,
            tc.tile_pool(name="outp", bufs=4) as opool,
            tc.tile_pool(name="psw", bufs=4, space="PSUM") as pwpool,
            tc.tile_pool(name="pso", bufs=2, space="PSUM") as popool,
        ):
            w_sb = cpool.tile([D, D], f32, tag="w")
            nc.sync.dma_start(out=w_sb[:], in_=wmat.ap())
            iota_sb = cpool.tile([P, nmm * P], bf16, tag="iota")
            nc.sync.dma_start(out=iota_sb[:], in_=iota.ap())
            il_sb = cpool.tile([P, n_lo // 16], mybir.dt.int16, tag="il")
            nc.sync.dma_start(out=il_sb[:], in_=idx_lo.ap())
            ih_sb = cpool.tile([P, n_hi // 16], mybir.dt.int16, tag="ih")
            nc.sync.dma_start(out=ih_sb[:], in_=idx_hi.ap())
            dlh_sb = cpool.tile([P, nw, nmm], bf16, tag="dlh")
            nc.sync.dma_start(out=dlh_sb[:], in_=dlh.ap())

            const_ab = [None, None]
            csel = None
            if parts == "constsel":
                csel = cpool.tile([P, P], bf16, tag="cselw")
                nc.vector.memset(csel[:], 0.25)
            if parts == "compute":
                ca = cpool.tile([P, cg * t_lo, D], f32, tag="cstga")
                cb = cpool.tile([P, cg * t_hi, D], f32, tag="cstgb")
                nc.vector.memset(ca[:], 0.5)
                nc.vector.memset(cb[:], 0.5)
                const_ab = [ca, cb]

            def body():
              for ch in range(nch):
                if parts == "compute":
                    sa_tile, sb_tile = const_ab
                else:
                    sa_tile = sapool.tile([P, cg * t_lo, D], f32, tag="stga")
                    sb_tile = sbpool.tile([P, cg * t_hi, D], f32, tag="stgb")
                if parts in ("all", "gather", "nodma", "gathercast",
                             "constsel"):
                    _emit_gathers(ch, sa_tile, sb_tile)
                if parts == "gathercast":
                    a_tile = apool.tile([P, cg * t_lo, D], bf16, tag="msga")
                    b_tile = bpool.tile([P, cg * t_hi, D], bf16, tag="msgb")
                    nc.scalar.copy(a_tile[:], sa_tile[:])
                    nc.scalar.copy(b_tile[:], sb_tile[:])
                if parts in ("all", "compute", "nodma", "constsel"):
                    _emit_compute(ch, sa_tile, sb_tile)

            def _emit_gathers(ch, sa_tile, sb_tile):
                # keep all 4 SWDGE queues busy concurrently: each gather's
                # effective rate is per-queue-stream limited, so split both
                # tables' chunks in half across two queues each
                def emit_split(tile_, tbl, idx_sb, ntiles, base_col, queues):
                    cut = ntiles // 2
                    parts_ = [(0, cut), (cut, ntiles)]
                    for (s, e), q in zip(parts_, queues):
                        n = (e - s) * P
                        nc.gpsimd.dma_gather(
                            tile_[:, s:e, :],
                            tbl.ap(),
                            idx_sb[:, base_col + s * 8 : base_col + e * 8],
                            n,
                            n,
                            D,
                            queue_num=q,
                            single_packet=single_packet,
                        )
                if nq == 2:
                    nc.gpsimd.dma_gather(
                        sa_tile[:], tbl_lo.ap(),
                        il_sb[:, ch * cg * t_lo * 8 : (ch + 1) * cg * t_lo * 8],
                        cg * t_lo * P, cg * t_lo * P, D,
                        queue_num=0, single_packet=single_packet,
                    )
                    nc.gpsimd.dma_gather(
                        sb_tile[:], tbl_hi.ap(),
                        ih_sb[:, ch * cg * t_hi * 8 : (ch + 1) * cg * t_hi * 8],
                        cg * t_hi * P, cg * t_hi * P, D,
                        queue_num=1, single_packet=single_packet,
                    )
                else:
                    # alternate the pair assignment so every ring carries
                    # the same bytes over a rep (lo chunks are 60% larger)
                    ql, qh = ((0, 2), (1, 3)) if ch % 2 == 0 else ((1, 3), (0, 2))
                    emit_split(sa_tile, tbl_lo, il_sb, cg * t_lo,
                               ch * cg * t_lo * 8, ql)
                    emit_split(sb_tile, tbl_hi, ih_sb, cg * t_hi,
                               ch * cg * t_hi * 8, qh)

            def _emit_compute(ch, sa_tile, sb_tile):
                # f32 staging -> bf16 messages: big table on scalar engine,
                # small one on DVE (balances the two engines)
                a_tile = apool.tile([P, cg * t_lo, D], bf16, tag="msga")
                b_tile = bpool.tile([P, cg * t_hi, D], bf16, tag="msgb")
                nc.scalar.copy(a_tile[:], sa_tile[:])
                nc.vector.tensor_scalar(
                    b_tile[:], sb_tile[:], 0.0, None, mybir.AluOpType.add
                )
                for wi in range(cg):
                    w = ch * cg + wi
                    if csel is not None:
                        sel_win = None
                    else:
                        # one DVE op builds the window's whole sel strip:
                        # sel[e, k*128+d] = (d == dloc[e, w, k])
                        sel_win = spool.tile([P, nmm * P], bf16, tag="sel")
                        nc.vector.tensor_tensor(
                            sel_win[:],
                            iota_sb[:],
                            dlh_sb[:, w, :, None].broadcast_to([P, nmm, P]),
                            mybir.AluOpType.is_equal,
                        )
                    psw = pwpool.tile([P, P], f32, tag="psw")
                    for k in range(nmm):
                        sel = csel if csel is not None else sel_win[:, k * P : (k + 1) * P]
                        if k < t_lo:
                            mt = a_tile[:, wi * t_lo + k, :]
                        else:
                            t = k - t_lo
                            mt = b_tile[:, wi * t_hi + t, :]
                        nc.tensor.matmul(
                            psw[:],
                            mt,
                            sel,
                            start=(k == 0),
                            stop=(k == nmm - 1),
                        )
                    # psw is aggT for this window: [dim, dst_local]
                    agg_t = gpool.tile([P, P], f32, tag="agg")
                    nc.scalar.copy(agg_t[:], psw[:])
                    pso = popool.tile([P, P], f32, tag="pso")
                    nc.tensor.matmul(
                        pso[:], agg_t[:], w_sb[:], start=True, stop=True
                    )
                    o_sb = opool.tile([P, D], f32, tag="out")
                    nc.scalar.activation(
                        o_sb[:], pso[:], mybir.ActivationFunctionType.Relu
                    )
                    if parts != "nodma":
                        nc.sync.dma_start(
                            out=out.ap()[w * P : (w + 1) * P, :], in_=o_sb[:]
                        )

            if bench_reps == 1:
                body()
            else:
                with tc.For_i(0, bench_reps, 1):
                    body()

    nc.compile()
    _NC_CACHE[key] = nc
    return nc


def _grid(bucket, mask, order_vals_idx, order_vals_dloc, t, nw=NW, n_cores=N_CORES):
    """Pack one src-half's edges into the fixed per-core slot grid."""
    nb = n_cores * nw
    b = bucket[mask]
    order = np.argsort(b, kind="stable")
    b_sorted = b[order]
    cnts = np.bincount(b_sorted, minlength=nb)
    starts = np.concatenate([[0], np.cumsum(cnts)[:-1]])
    rank = np.arange(len(b_sorted)) - starts[b_sorted]
    spb = t * P
    n = nw * spb
    flat_idx = np.zeros((n_cores, n), dtype=np.int16)
    flat_dloc = np.full((n_cores, n), -1.0, dtype=np.float32)
    c = b_sorted // nw
    wloc = b_sorted % nw
    pos = wloc * spb + rank
    flat_idx[c, pos] = order_vals_idx[mask][order]
    flat_dloc[c, pos] = order_vals_dloc[mask][order]
    idx16 = flat_idx.reshape(n_cores, n // 16, 16).transpose(0, 2, 1)
    idx16 = np.ascontiguousarray(np.tile(idx16, (1, 8, 1)))
    dl = np.ascontiguousarray(flat_dloc.reshape(n_cores, nw * t, P).transpose(0, 2, 1))
    return idx16, dl


def balance_windows(src, dst, n_nodes, nwtot, s_split):
    """Returns (win_of[d], lane_of[d], cnt_lo[w], cnt_hi[w])."""
    is_lo = src < s_split
    cl = np.bincount(dst[is_lo], minlength=n_nodes).astype(np.int64)
    ch = np.bincount(dst[~is_lo], minlength=n_nodes).astype(np.int64)
    capL = float(cl.sum()) / nwtot
    capH = float(ch.sum()) / nwtot

    order = np.argsort(-(cl / capL + ch / capH), kind="stable")
    pad = nwtot * P - n_nodes
    order_p = np.concatenate([order, np.full(pad, -1)])
    suml = np.zeros(nwtot)
    sumh = np.zeros(nwtot)
    nfill = np.zeros(nwtot, np.int64)
    win_of = np.empty(n_nodes, np.int32)
    lane_of = np.empty(n_nodes, np.int32)
    for r in range(P):
        batch = order_p[r * nwtot : (r + 1) * nwtot]
        wload = np.maximum(suml / capL, sumh / capH) + 0.001 * (
            suml / capL + sumh / capH
        )
        worder = np.argsort(wload, kind="stable")
        for i, d in enumerate(batch):
            w = worder[i]
            if d >= 0:
                win_of[d] = w
                lane_of[d] = nfill[w]
                suml[w] += cl[d]
                sumh[w] += ch[d]
            nfill[w] += 1
    cnt_lo = np.zeros(nwtot, np.int64)
    cnt_hi = np.zeros(nwtot, np.int64)
    np.add.at(cnt_lo, win_of, cl)
    np.add.at(cnt_hi, win_of, ch)
    return win_of, lane_of, cnt_lo, cnt_hi


def _pack(x, edge_index, W):
    import ml_dtypes
    bf = ml_dtypes.bfloat16

    src = edge_index[0].astype(np.int64)
    dst = edge_index[1].astype(np.int64)

    win_of, lane_of, cw_lo, cw_hi = balance_windows(
        src, dst, N_NODES, N_CORES * NW, S_SPLIT
    )
    bucket = win_of[dst]
    dloc_all = lane_of[dst].astype(np.float32)
    is_hi = src >= S_SPLIT

    t_lo = max(1, int(np.ceil(cw_lo.max() / P)))
    t_hi = max(1, int(np.ceil(cw_hi.max() / P)))

    idx_val_lo = (src + 1).astype(np.int16, casting="unsafe")
    idx_val_hi = (src - S_SPLIT + 1).astype(np.int16, casting="unsafe")
    idx16_lo, dloc_lo = _grid(bucket, ~is_hi, idx_val_lo, dloc_all, t_lo)
    idx16_hi, dloc_hi = _grid(bucket, is_hi, idx_val_hi, dloc_all, t_hi)
    nmm = t_lo + t_hi

    tbl_lo = np.zeros((LO_ROWS, D), np.float32)
    tbl_lo[1:] = x[:S_SPLIT]
    tbl_hi = np.zeros((HI_ROWS, D), np.float32)
    tbl_hi[1:] = x[S_SPLIT:]
    iota = np.ascontiguousarray(
        np.tile(np.arange(P, dtype=np.float32), (P, nmm))
    ).astype(bf)
    # dlh[c][e, w*nmm + k]: dst_local of slot (w, tile k) — lo tiles
    # first, then hi tiles
    dlh = np.empty((N_CORES, P, NW, nmm), np.float32)
    dlh[:, :, :, :t_lo] = dloc_lo.reshape(N_CORES, P, NW, t_lo)
    dlh[:, :, :, t_lo:] = dloc_hi.reshape(N_CORES, P, NW, t_hi)
    dlh = np.ascontiguousarray(dlh.reshape(N_CORES, P, NW * nmm)).astype(bf)

    in_maps = []
    for c in range(N_CORES):
        in_maps.append(
            {
                "tbl_lo": tbl_lo,
                "tbl_hi": tbl_hi,
                "idx_lo": idx16_lo[c],
                "idx_hi": idx16_hi[c],
                "dlh": dlh[c],
                "wmat": W,
                "iota": iota,
            }
        )
    return t_lo, t_hi, in_maps, (win_of, lane_of)


def kernel(x, edge_index, W):
    x = np.asarray(x, dtype=np.float32)
    edge_index = np.asarray(edge_index)
    W = np.asarray(W, dtype=np.float32)
    assert x.shape == (N_NODES, D) and W.shape == (D, D)

    t_lo, t_hi, in_maps, (win_of, lane_of) = _pack(x, edge_index, W)
    nc = _build_nc(t_lo, t_hi)

    from concourse.bass_utils import run_bass_kernel_spmd

    res = run_bass_kernel_spmd(nc, in_maps, core_ids=list(range(N_CORES)))
    global _LAST_RUN, _LAST_CAPS
    _LAST_RUN = (nc, in_maps)
    _LAST_CAPS = (t_lo, t_hi)
    allout = np.stack([res.results[c]["out"] for c in range(N_CORES)])
    d = np.arange(N_NODES)
    full = allout[win_of // NW, (win_of % NW) * P + lane_of]
    return np.ascontiguousarray(full.astype(np.float32))


_LAST_RUN = None
_LAST_CAPS = None
